# revision 24
# baseline (speedup 1.0000x reference)
"""Multi-head self-attention with ALiBi + RoPE, tensor-parallel over 8 NeuronCores.

Sharding: heads split across cores (2 heads/core). Each core computes its
heads' QKV projection, RoPE, attention (scores kept transposed [s, t] so no
PE transposes are needed), and a partial out-projection over its 256
channels. The 8 partial outputs are summed on the host.

Hardcoded problem shape: B=2, T=2048, C=2048, H=16, D=128.
"""

import sys

for _p in ('/opt/trn_rl_repo', '/root/.axon_site/_ro/trn_rl_repo'):
    if _p not in sys.path:
        sys.path.insert(0, _p)

import numpy as np

import bass_rust
import concourse.bass as bass
import concourse.tile as tile
import concourse.mybir as mybir

B, T, C, H = 2, 2048, 2048, 16
D = C // H            # 128
NCORES = 8
HLOC = H // NCORES    # heads per core = 2
ROPE_BASE = 10000.0
SCALE = 1.0 / np.sqrt(D)

F32 = mybir.dt.float32
F32R = mybir.dt.float32r
BT = B * T            # 4096 rows


def _r(ap):
    return ap.bitcast(F32R)


def _f(ap):
    return ap.bitcast(F32)


def split_excess_waits(nc, limit=1):
    """walrus CTRL codegen rejects >1 sem wait per instruction; move excess
    waits onto preceding NoOps on the same engine."""
    import copy as _copy
    ctrl_types = (bass_rust.InstDrain, bass_rust.InstNoOp, bass_rust.InstHalt,
                  bass_rust.InstEventSemaphore, bass_rust.InstAllEngineBarrier)
    ctr = 0
    for f in nc.m.functions:
        new_blocks = []
        for b in f.blocks:
            out = []
            changed = False
            for inst in b.instructions:
                si = inst.sync_info
                lim = limit
                if si is not None and si.on_wait and len(si.on_wait) > lim:
                    waits = list(si.on_wait)
                    excess, keep = waits[:-lim], waits[-lim:]
                    for i in range(0, len(excess), limit):
                        ctr += 1
                        nop = bass_rust.InstNoOp(
                            name=f"I-waitsplit-{ctr}", engine=inst.engine)
                        nop.sync_info = mybir.SyncInfo(
                            on_wait=excess[i:i + limit], on_update=[])
                        out.append(nop)
                    inst.sync_info = mybir.SyncInfo(
                        on_wait=keep, on_update=list(si.on_update or []))
                    changed = True
                out.append(inst)
            new_blocks.append(_copy.replace(b, instructions=out) if changed else b)
        f.blocks.clear()
        for nb in new_blocks:
            f.blocks.append(nb)
    return ctr


def build_bass():
    nc = bass.Bass(enable_partition_id=False)

    xT = nc.dram_tensor("xT", [C, BT], F32R, kind="ExternalInput")
    wqkT = nc.dram_tensor("wqkT", [C, 4 * D], F32R, kind="ExternalInput")
    wvT = nc.dram_tensor("wvT", [C, HLOC * D], F32R, kind="ExternalInput")
    prot = nc.dram_tensor("prot", [D, D], F32R, kind="ExternalInput")
    identw = nc.dram_tensor("identw", [128, 128], F32R, kind="ExternalInput")
    cq = nc.dram_tensor("cq", [D, BT], F32, kind="ExternalInput")
    sq = nc.dram_tensor("sq", [D, BT], F32, kind="ExternalInput")
    ck = nc.dram_tensor("ck", [D, BT], F32, kind="ExternalInput")
    sk = nc.dram_tensor("sk", [D, BT], F32, kind="ExternalInput")
    biasT = nc.dram_tensor("biasT", [HLOC, T, T], F32R, kind="ExternalInput")
    woT = nc.dram_tensor("woT", [HLOC * D, C], F32R, kind="ExternalInput")
    out = nc.dram_tensor("out", [BT, C], F32, kind="ExternalOutput")

    NCC = C // 128        # 16 contraction chunks
    NTG = BT // 256       # 16 t-groups in phase 1
    NSC = T // 128        # 16 s-chunks per batch

    with tile.TileContext(nc) as tc:
        with (
            tc.tile_pool(name="persist", bufs=1) as pp,
            tc.tile_pool(name="qkv", bufs=1) as qkvp,
        ):
            prot_sb = pp.tile([D, D], F32R, tag="prot", name="prot_sb")
            nc.sync.dma_start(prot_sb[:], prot[:])
            ones_sb = pp.tile([128, 1], F32, tag="ones", name="ones_sb")
            nc.vector.memset(ones_sb[:], 1.0)
            ident_sb = pp.tile([128, 128], F32R, tag="ident", name="ident_sb")
            nc.sync.dma_start(ident_sb[:], identw[:])

            # q0 q1 k0 k1 transposed [d, t]; v natural [t-in, chunk, f]
            qk_t = [qkvp.tile([D, BT], F32R, tag=f"qk{i}", name=f"qk{i}")
                    for i in range(4)]
            v_sb = qkvp.tile([128, BT // 128, HLOC * D], F32R, tag="v",
                             name="v_sb")

            # ---------- phase 1: QKV projection ----------
            with (
                tc.tile_pool(name="w1", bufs=1) as w1p,
                tc.tile_pool(name="xt", bufs=2) as xtp,
                tc.tile_pool(name="ps1", bufs=4, space="PSUM") as ps1,
            ):
                wqk_sb = w1p.tile([128, NCC, 4 * D], F32R, tag="wqk",
                                  name="wqk_sb")
                wv_sb = w1p.tile([128, NCC, HLOC * D], F32R, tag="wv",
                                 name="wv_sb")
                # chunk 0 of the weights first, then tg0's activations, so
                # the first matmul can start ~4us in; bulk loads after.
                nc.sync.dma_start(
                    wqk_sb[:, 0:4, :],
                    wqkT[0:512, :].rearrange("(k p) f -> p k f", p=128))

                def load_tg(tg):
                    sl = slice(tg * 256, (tg + 1) * 256)
                    xt = xtp.tile([128, NCC, 256], F32R, tag="xt", name="xt")
                    for xi in range(4):
                        nc.sync.dma_start(
                            xt[:, xi * 4:(xi + 1) * 4, :],
                            xT[xi * 512:(xi + 1) * 512, sl].rearrange(
                                "(k p) t -> p k t", p=128))
                    cqt = xtp.tile([D, 256], F32, tag="cqt", name="cqt")
                    sqt = xtp.tile([D, 256], F32, tag="sqt", name="sqt")
                    ckt = xtp.tile([D, 256], F32, tag="ckt", name="ckt")
                    skt = xtp.tile([D, 256], F32, tag="skt", name="skt")
                    nc.sync.dma_start(cqt[:], cq[:, sl])
                    nc.sync.dma_start(sqt[:], sq[:, sl])
                    nc.sync.dma_start(ckt[:], ck[:, sl])
                    nc.sync.dma_start(skt[:], sk[:, sl])
                    return xt, {0: (cqt, sqt), 1: (cqt, sqt),
                                2: (ckt, skt), 3: (ckt, skt)}

                # interleave remaining weight chunks with tg0 activations so
                # the fb0 accumulation is never starved mid-K.
                sl0 = slice(0, 256)
                xt0 = xtp.tile([128, NCC, 256], F32R, tag="xt", name="xt")
                for xi in range(4):
                    nc.sync.dma_start(
                        xt0[:, xi * 4:(xi + 1) * 4, :],
                        xT[xi * 512:(xi + 1) * 512, sl0].rearrange(
                            "(k p) t -> p k t", p=128))
                    if xi < 3:
                        nc.sync.dma_start(
                            wqk_sb[:, (xi + 1) * 4:(xi + 2) * 4, :],
                            wqkT[(xi + 1) * 512:(xi + 2) * 512, :].rearrange(
                                "(k p) f -> p k f", p=128))
                cqt0 = xtp.tile([D, 256], F32, tag="cqt", name="cqt")
                sqt0 = xtp.tile([D, 256], F32, tag="sqt", name="sqt")
                ckt0 = xtp.tile([D, 256], F32, tag="ckt", name="ckt")
                skt0 = xtp.tile([D, 256], F32, tag="skt", name="skt")
                nc.sync.dma_start(cqt0[:], cq[:, sl0])
                nc.sync.dma_start(sqt0[:], sq[:, sl0])
                nc.sync.dma_start(ckt0[:], ck[:, sl0])
                nc.sync.dma_start(skt0[:], sk[:, sl0])
                tg0_tiles = (xt0, {0: (cqt0, sqt0), 1: (cqt0, sqt0),
                                   2: (ckt0, skt0), 3: (ckt0, skt0)})
                nc.sync.dma_start(
                    wv_sb[:], wvT[:].rearrange("(k p) f -> p k f", p=128))

                for tg in range(NTG):
                    sl = slice(tg * 256, (tg + 1) * 256)
                    xt, cs_t = tg0_tiles if tg == 0 else load_tg(tg)
                    for fb in range(4):       # q0 q1 k0 k1
                        ps = ps1.tile([128, 256], F32, tag="ps1", name="ps")
                        for cc in range(NCC):
                            nc.tensor.matmul(
                                ps[:],
                                wqk_sb[:, cc, fb * 128:(fb + 1) * 128],
                                xt[:, cc, :],
                                start=(cc == 0), stop=(cc == NCC - 1))
                        qslice = qk_t[fb][:, sl]
                        nc.vector.tensor_copy(qslice, ps[:])
                        # RoPE on this 256-wide slice
                        pr = ps1.tile([D, 256], F32, tag="rot", name="pr",
                                      bufs=2)
                        nc.tensor.matmul(pr[:], prot_sb[:], qslice,
                                         start=True, stop=True)
                        ct, st_ = cs_t[fb]
                        t1 = xtp.tile([D, 256], F32, tag="t1", name="t1")
                        t2 = xtp.tile([D, 256], F32, tag="t2", name="t2")
                        nc.vector.tensor_mul(t1[:], pr[:], st_[:])
                        nc.vector.tensor_mul(t2[:], _f(qslice), ct[:])
                        nc.vector.tensor_add(qslice, t1[:], t2[:])
                    for tb in range(2):       # v natural
                        ps = ps1.tile([128, HLOC * D], F32, tag="ps1",
                                      name="ps")
                        for cc in range(NCC):
                            nc.tensor.matmul(
                                ps[:],
                                xt[:, cc, tb * 128:(tb + 1) * 128],
                                wv_sb[:, cc, :],
                                start=(cc == 0), stop=(cc == NCC - 1))
                        nc.scalar.copy(v_sb[:, tg * 2 + tb, :], ps[:])

            # ---------- phases 2+3 ----------
            with tc.tile_pool(name="aop", bufs=1) as aop:
                ao_t = [aop.tile([D, BT], F32R, tag=f"ao{h}", name=f"ao{h}")
                        for h in range(HLOC)]
                wo_sb = aop.tile([128, HLOC, C], F32R, tag="wo",
                                 name="wo_sb")

                # phase 2: attention
                with (
                    tc.tile_pool(name="att", bufs=3) as ap_,
                    tc.tile_pool(name="lp", bufs=2) as lp,
                    tc.tile_pool(name="ldram", bufs=2, space="DRAM") as ldp,
                    tc.tile_pool(name="pss", bufs=2, space="PSUM") as pss,
                    tc.tile_pool(name="pso", bufs=1, space="PSUM") as pso,
                ):
                    for h in range(HLOC):
                        if h == HLOC - 1:
                            nc.sync.dma_start(
                                wo_sb[:],
                                woT[:].rearrange("(h p) o -> p h o", p=128))
                        q_t, k_t = qk_t[h], qk_t[2 + h]
                        for tg2 in range(2):      # 1024-wide column groups
                            po = [pso.tile([D, 1024], F32, tag=f"po{b}",
                                           name=f"po{b}") for b in range(B)]
                            lacc = [lp.tile([128, 1024], F32, tag=f"l{b}",
                                            name=f"l{b}") for b in range(B)]
                            for sc in range(NSC):
                                bt = ap_.tile([128, 1024], F32R, tag="bias",
                                              name="bt", bufs=4)
                                nc.sync.dma_start(
                                    bt[:],
                                    biasT[h, sc * 128:(sc + 1) * 128,
                                          tg2 * 1024:(tg2 + 1) * 1024])
                                for b in range(B):
                                    t0 = b * T + tg2 * 1024
                                    ps = pss.tile([128, 1024], F32, tag="ps",
                                                  name="ps")
                                    for hf in range(2):
                                        nc.tensor.matmul(
                                            ps[:, hf * 512:(hf + 1) * 512],
                                            ident_sb[:],
                                            bt[:, hf * 512:(hf + 1) * 512],
                                            start=True, stop=False,
                                            skip_group_check=True)
                                        nc.tensor.matmul(
                                            ps[:, hf * 512:(hf + 1) * 512],
                                            k_t[:, b * T + sc * 128:
                                                b * T + (sc + 1) * 128],
                                            q_t[:, t0 + hf * 512:
                                                t0 + (hf + 1) * 512],
                                            start=False, stop=True,
                                            skip_group_check=True)
                                    pe = ap_.tile([128, 1024], F32R, tag="pe",
                                                  name="pe", bufs=4)
                                    nc.scalar.activation(
                                        pe[:], ps[:],
                                        mybir.ActivationFunctionType.Exp)
                                    eng = nc.vector if b == 0 else nc.gpsimd
                                    if sc == 0:
                                        eng.tensor_copy(lacc[b][:], _f(pe[:]))
                                    else:
                                        eng.tensor_add(lacc[b][:], lacc[b][:],
                                                       _f(pe[:]))
                                    for hf in range(2):
                                        nc.tensor.matmul(
                                            po[b][:, hf * 512:(hf + 1) * 512],
                                            v_sb[:, b * NSC + sc,
                                                 h * 128:(h + 1) * 128],
                                            pe[:, hf * 512:(hf + 1) * 512],
                                            start=(sc == 0),
                                            stop=(sc == NSC - 1),
                                            skip_group_check=True)
                            for b in range(B):
                                t0 = b * T + tg2 * 1024
                                ao_sl = ao_t[h][:, t0:t0 + 1024]
                                # evict unnormalized so po frees fast
                                nc.scalar.copy(ao_sl, po[b][:])
                                psl = pss.tile([1, 1024], F32, tag="ps",
                                               name="psl")
                                for hf in range(2):
                                    nc.tensor.matmul(
                                        psl[:, hf * 512:(hf + 1) * 512],
                                        ones_sb[:],
                                        lacc[b][:, hf * 512:(hf + 1) * 512],
                                        start=True, stop=True,
                                        skip_group_check=True)
                                linv = lacc[b][0:1, :]
                                nc.vector.reciprocal(linv, psl[:])
                                ldr = ldp.tile([1, 1024], F32, tag="ldr",
                                               name="ldr")
                                nc.sync.dma_start(ldr[:], linv)
                                linb = ap_.tile([128, 1024], F32, tag="pe",
                                                name="linb", bufs=4)
                                nc.sync.dma_start(
                                    linb[:], ldr[:].broadcast_to((128, 1024)))
                                nc.vector.tensor_mul(ao_sl, _f(ao_sl),
                                                     linb[:])

                    # phase 3: partial out-projection (same pool scope;
                    # psum via the po tags, no pool boundary barrier)
                    for ts in range(BT // 128):
                        r0 = ts * 128
                        for oh in range(2):
                            pt = pso.tile([D, 1024], F32, tag=f"po{oh}",
                                          name="pt")
                            for oc2 in range(2):
                                o0 = oh * 1024 + oc2 * 512
                                for hh in range(HLOC):
                                    nc.tensor.matmul(
                                        pt[:, oc2 * 512:(oc2 + 1) * 512],
                                        ao_t[hh][:, r0:r0 + 128],
                                        wo_sb[:, hh, o0:o0 + 512],
                                        start=(hh == 0),
                                        stop=(hh == HLOC - 1),
                                        skip_group_check=True)
                            stg = ap_.tile([128, 1024], F32, tag="stg",
                                           name="stg", bufs=3)
                            if (ts + oh) % 2 == 0:
                                nc.scalar.copy(stg[:], pt[:])
                            else:
                                nc.vector.tensor_copy(stg[:], pt[:])
                            nc.sync.dma_start(
                                out[r0:r0 + 128,
                                    oh * 1024:(oh + 1) * 1024],
                                stg[:])

    split_excess_waits(nc, limit=1)
    return nc


def prep_inputs(x, attn_mask, alibi_bias, Wqkv, Wout):
    """Host-side sharding: returns in_maps (list of 8 dicts)."""
    x = np.asarray(x, np.float32)
    attn_mask = np.asarray(attn_mask, np.float32)
    alibi_bias = np.asarray(alibi_bias, np.float32)
    Wqkv = np.asarray(Wqkv, np.float32)
    Wout = np.asarray(Wout, np.float32)

    xT = np.ascontiguousarray(x.reshape(BT, C).T)          # [C, BT]

    inv_freq = 1.0 / (ROPE_BASE ** (np.arange(0, D, 2, dtype=np.float32) / D))
    pos = np.arange(T, dtype=np.float32)
    freqs = np.einsum('i,j->ij', pos, inv_freq)
    emb = np.concatenate([freqs, freqs], axis=-1)          # [T, D]
    cosT = np.ascontiguousarray(np.cos(emb).T.astype(np.float32))  # [D, T]
    sinT = np.ascontiguousarray(np.sin(emb).T.astype(np.float32))
    cosT2 = np.concatenate([cosT, cosT], axis=1)           # [D, BT]
    sinT2 = np.concatenate([sinT, sinT], axis=1)
    cq = np.ascontiguousarray(cosT2 * SCALE)
    sq = np.ascontiguousarray(sinT2 * SCALE)
    ck = np.ascontiguousarray(cosT2)
    sk = np.ascontiguousarray(sinT2)

    P = np.zeros((D, D), np.float32)
    P[np.arange(64), np.arange(64) + 64] = -1.0
    P[np.arange(64) + 64, np.arange(64)] = 1.0
    protT = np.ascontiguousarray(P.T)

    Wq, Wk, Wv = Wqkv[0:C], Wqkv[C:2 * C], Wqkv[2 * C:3 * C]
    # bias per head, transposed: biasT_h[s, t] = mask[t, s] + alibi[h, t, s]
    biasT_all = np.ascontiguousarray(
        (attn_mask[None] + alibi_bias).transpose(0, 2, 1))

    in_maps = []
    for c in range(NCORES):
        lo, hi = c * HLOC * D, (c + 1) * HLOC * D
        qk_rows = np.concatenate([Wq[lo:hi], Wk[lo:hi]], axis=0)  # [512, C]
        in_maps.append({
            "xT": xT,
            "wqkT": np.ascontiguousarray(qk_rows.T),
            "wvT": np.ascontiguousarray(Wv[lo:hi].T),
            "prot": protT,
            "identw": np.eye(128, dtype=np.float32),
            "cq": cq, "sq": sq, "ck": ck, "sk": sk,
            "biasT": np.ascontiguousarray(biasT_all[c * HLOC:(c + 1) * HLOC]),
            "woT": np.ascontiguousarray(Wout[:, lo:hi].T),
        })
    return in_maps


# ---------------------------------------------------------------------------
# PJRT runner (adapted from concourse.bass2jax.run_bass_via_pjrt, without
# output-buffer donation so the jitted callable can be re-run for timing).
# ---------------------------------------------------------------------------
_CACHE = {}


def _get_runner():
    if "runner" in _CACHE:
        return _CACHE["runner"]

    import jax
    from jax.sharding import Mesh, PartitionSpec
    from jax.experimental.shard_map import shard_map
    from concourse.bass2jax import _bass_exec_p, install_neuronx_cc_hook

    install_neuronx_cc_hook()
    nc = build_bass()

    in_names, out_names, out_avals, zero_outs = [], [], [], []
    for alloc in nc.m.functions[0].allocations:
        if not isinstance(alloc, mybir.MemoryLocationSet):
            continue
        name = alloc.memorylocations[0].name
        if alloc.kind == "ExternalInput":
            in_names.append(name)
        elif alloc.kind == "ExternalOutput":
            out_names.append(name)
            shape = tuple(alloc.tensor_shape)
            dtype = mybir.dt.np(alloc.dtype)
            out_avals.append(jax.core.ShapedArray(shape, dtype))
            zero_outs.append(np.zeros(shape, dtype))
    n_params = len(in_names)
    all_names = in_names + out_names

    def _body(*args):
        outs = _bass_exec_p.bind(
            *args,
            out_avals=tuple(out_avals),
            in_names=tuple(all_names),
            out_names=tuple(out_names),
            lowering_input_output_aliases=(),
            sim_require_finite=True,
            sim_require_nnan=True,
            nc=nc,
        )
        return tuple(outs)

    devices = jax.devices()[:NCORES]
    mesh = Mesh(np.asarray(devices), ("core",))
    n_all = n_params + len(out_names)
    sharded = jax.jit(
        shard_map(
            _body, mesh=mesh,
            in_specs=(PartitionSpec("core"),) * n_all,
            out_specs=(PartitionSpec("core"),) * len(out_names),
            check_rep=False,
        ),
        keep_unused=True,
    )
    _CACHE["nc_obj"] = nc
    _CACHE["runner"] = (sharded, in_names, out_names, out_avals, zero_outs)
    return _CACHE["runner"]


def _run_device(in_maps):
    import jax
    sharded, in_names, out_names, out_avals, zero_outs = _get_runner()
    concat_in = [
        np.concatenate([in_maps[c][n] for c in range(NCORES)], axis=0)
        for n in in_names
    ]
    concat_zero = [
        np.zeros((NCORES * z.shape[0], *z.shape[1:]), z.dtype)
        for z in zero_outs
    ]
    args = [jax.device_put(a) for a in concat_in + concat_zero]
    _CACHE["last_args"] = args
    out_arrs = sharded(*args)
    out_arrs = [np.asarray(o) for o in out_arrs]
    return [
        {n: out_arrs[i].reshape(NCORES, *out_avals[i].shape)[c]
         for i, n in enumerate(out_names)}
        for c in range(NCORES)
    ]


def bench(n=10):
    """Re-run the cached jitted fn on the last inputs; returns per-call
    wall seconds. Includes dispatch/tunnel overhead."""
    import time as _time
    sharded = _CACHE["runner"][0]
    args = _CACHE["last_args"]
    times = []
    for _ in range(n):
        t0 = _time.perf_counter()
        res = sharded(*args)
        for r in res:
            r.block_until_ready()
        times.append(_time.perf_counter() - t0)
    return times


def kernel(x, attn_mask, alibi_bias, Wqkv, Wout):
    in_maps = prep_inputs(x, attn_mask, alibi_bias, Wqkv, Wout)
    results = _run_device(in_maps)
    acc = results[0]["out"].astype(np.float32).copy()
    for c in range(1, NCORES):
        acc += results[c]["out"]
    return acc.reshape(B, T, C)



def bench_async(ks=(1, 8, 16), n=4):
    """Queue k async dispatches of the cached jitted fn, block once.
    Marginal device time ~ (T(k2) - T(k1)) / (k2 - k1)."""
    import time as _time
    sharded = _CACHE["runner"][0]
    args = _CACHE["last_args"]
    out = {}
    for k in ks:
        best = float("inf")
        for _ in range(n):
            t0 = _time.perf_counter()
            rs = []
            for _i in range(k):
                rs.append(sharded(*args))
            for x in rs[-1]:
                x.block_until_ready()
            best = min(best, _time.perf_counter() - t0)
        out[k] = best
    return out


# revision 28
# speedup vs baseline: 1.0096x; 1.0096x over previous
"""Multi-head self-attention with ALiBi + RoPE, tensor-parallel over 8 NeuronCores.

Sharding: heads split across cores (2 heads/core). Each core computes its
heads' QKV projection, RoPE, attention (scores kept transposed [s, t] so no
PE transposes are needed), and a partial out-projection over its 256
channels. The 8 partial outputs are summed on the host.

Hardcoded problem shape: B=2, T=2048, C=2048, H=16, D=128.
"""

import sys

for _p in ('/opt/trn_rl_repo', '/root/.axon_site/_ro/trn_rl_repo'):
    if _p not in sys.path:
        sys.path.insert(0, _p)

import numpy as np

import bass_rust
import concourse.bass as bass
import concourse.tile as tile
import concourse.mybir as mybir

B, T, C, H = 2, 2048, 2048, 16
D = C // H            # 128
NCORES = 8
HLOC = H // NCORES    # heads per core = 2
ROPE_BASE = 10000.0
SCALE = 1.0 / np.sqrt(D)

F32 = mybir.dt.float32
F32R = mybir.dt.float32r
BT = B * T            # 4096 rows


def _r(ap):
    return ap.bitcast(F32R)


def _f(ap):
    return ap.bitcast(F32)


def split_excess_waits(nc, limit=1):
    """walrus CTRL codegen rejects >1 sem wait per instruction; move excess
    waits onto preceding NoOps on the same engine."""
    import copy as _copy
    ctrl_types = (bass_rust.InstDrain, bass_rust.InstNoOp, bass_rust.InstHalt,
                  bass_rust.InstEventSemaphore, bass_rust.InstAllEngineBarrier)
    ctr = 0
    for f in nc.m.functions:
        new_blocks = []
        for b in f.blocks:
            out = []
            changed = False
            for inst in b.instructions:
                si = inst.sync_info
                lim = limit
                if si is not None and si.on_wait and len(si.on_wait) > lim:
                    waits = list(si.on_wait)
                    excess, keep = waits[:-lim], waits[-lim:]
                    for i in range(0, len(excess), limit):
                        ctr += 1
                        nop = bass_rust.InstNoOp(
                            name=f"I-waitsplit-{ctr}", engine=inst.engine)
                        nop.sync_info = mybir.SyncInfo(
                            on_wait=excess[i:i + limit], on_update=[])
                        out.append(nop)
                    inst.sync_info = mybir.SyncInfo(
                        on_wait=keep, on_update=list(si.on_update or []))
                    changed = True
                out.append(inst)
            new_blocks.append(_copy.replace(b, instructions=out) if changed else b)
        f.blocks.clear()
        for nb in new_blocks:
            f.blocks.append(nb)
    return ctr


def build_bass():
    nc = bass.Bass(enable_partition_id=False)

    xT = nc.dram_tensor("xT", [C, BT], F32R, kind="ExternalInput")
    wqkT = nc.dram_tensor("wqkT", [C, 4 * D], F32R, kind="ExternalInput")
    wvT = nc.dram_tensor("wvT", [C, HLOC * D], F32R, kind="ExternalInput")
    prot = nc.dram_tensor("prot", [D, D], F32R, kind="ExternalInput")
    identw = nc.dram_tensor("identw", [128, 128], F32R, kind="ExternalInput")
    onesw = nc.dram_tensor("onesw", [128, 1], F32R, kind="ExternalInput")
    cq = nc.dram_tensor("cq", [D, BT], F32, kind="ExternalInput")
    sq = nc.dram_tensor("sq", [D, BT], F32, kind="ExternalInput")
    ck = nc.dram_tensor("ck", [D, BT], F32, kind="ExternalInput")
    sk = nc.dram_tensor("sk", [D, BT], F32, kind="ExternalInput")
    biasT = nc.dram_tensor("biasT", [HLOC, T, T], F32R, kind="ExternalInput")
    woT = nc.dram_tensor("woT", [HLOC * D, C], F32R, kind="ExternalInput")
    out = nc.dram_tensor("out", [BT, C], F32, kind="ExternalOutput")

    NCC = C // 128        # 16 contraction chunks
    NTG = BT // 256       # 16 t-groups in phase 1
    NSC = T // 128        # 16 s-chunks per batch

    with tile.TileContext(nc) as tc:
        with (
            tc.tile_pool(name="persist", bufs=1) as pp,
            tc.tile_pool(name="qkv", bufs=1) as qkvp,
        ):
            prot_sb = pp.tile([D, D], F32R, tag="prot", name="prot_sb")
            nc.sync.dma_start(prot_sb[:], prot[:])
            ones_sb = pp.tile([128, 1], F32R, tag="ones", name="ones_sb")
            nc.sync.dma_start(ones_sb[:], onesw[:])
            ident_sb = pp.tile([128, 128], F32R, tag="ident", name="ident_sb")
            nc.sync.dma_start(ident_sb[:], identw[:])

            # q0 q1 k0 k1 transposed [d, t]; v natural [t-in, chunk, f]
            qk_t = [qkvp.tile([D, BT], F32R, tag=f"qk{i}", name=f"qk{i}")
                    for i in range(4)]
            v_sb = qkvp.tile([128, BT // 128, HLOC * D], F32R, tag="v",
                             name="v_sb")

            # ---------- phase 1: QKV projection ----------
            with (
                tc.tile_pool(name="w1", bufs=1) as w1p,
                tc.tile_pool(name="xt", bufs=2) as xtp,
                tc.tile_pool(name="ps1", bufs=4, space="PSUM") as ps1,
            ):
                wqk_sb = w1p.tile([128, NCC, 4 * D], F32R, tag="wqk",
                                  name="wqk_sb")
                wv_sb = w1p.tile([128, NCC, HLOC * D], F32R, tag="wv",
                                 name="wv_sb")
                # chunk 0 of the weights first, then tg0's activations, so
                # the first matmul can start ~4us in; bulk loads after.
                nc.sync.dma_start(
                    wqk_sb[:, 0:4, :],
                    wqkT[0:512, :].rearrange("(k p) f -> p k f", p=128))

                def load_tg(tg):
                    sl = slice(tg * 256, (tg + 1) * 256)
                    xt = xtp.tile([128, NCC, 256], F32R, tag="xt", name="xt")
                    for xi in range(4):
                        nc.sync.dma_start(
                            xt[:, xi * 4:(xi + 1) * 4, :],
                            xT[xi * 512:(xi + 1) * 512, sl].rearrange(
                                "(k p) t -> p k t", p=128))
                    cqt = xtp.tile([D, 256], F32, tag="cqt", name="cqt")
                    sqt = xtp.tile([D, 256], F32, tag="sqt", name="sqt")
                    ckt = xtp.tile([D, 256], F32, tag="ckt", name="ckt")
                    skt = xtp.tile([D, 256], F32, tag="skt", name="skt")
                    nc.sync.dma_start(cqt[:], cq[:, sl])
                    nc.sync.dma_start(sqt[:], sq[:, sl])
                    nc.sync.dma_start(ckt[:], ck[:, sl])
                    nc.sync.dma_start(skt[:], sk[:, sl])
                    return xt, {0: (cqt, sqt), 1: (cqt, sqt),
                                2: (ckt, skt), 3: (ckt, skt)}

                # interleave remaining weight chunks with tg0 activations so
                # the fb0 accumulation is never starved mid-K.
                sl0 = slice(0, 256)
                xt0 = xtp.tile([128, NCC, 256], F32R, tag="xt", name="xt")
                for xi in range(4):
                    nc.sync.dma_start(
                        xt0[:, xi * 4:(xi + 1) * 4, :],
                        xT[xi * 512:(xi + 1) * 512, sl0].rearrange(
                            "(k p) t -> p k t", p=128))
                    if xi < 3:
                        nc.sync.dma_start(
                            wqk_sb[:, (xi + 1) * 4:(xi + 2) * 4, :],
                            wqkT[(xi + 1) * 512:(xi + 2) * 512, :].rearrange(
                                "(k p) f -> p k f", p=128))
                cqt0 = xtp.tile([D, 256], F32, tag="cqt", name="cqt")
                sqt0 = xtp.tile([D, 256], F32, tag="sqt", name="sqt")
                ckt0 = xtp.tile([D, 256], F32, tag="ckt", name="ckt")
                skt0 = xtp.tile([D, 256], F32, tag="skt", name="skt")
                nc.sync.dma_start(cqt0[:], cq[:, sl0])
                nc.sync.dma_start(sqt0[:], sq[:, sl0])
                nc.sync.dma_start(ckt0[:], ck[:, sl0])
                nc.sync.dma_start(skt0[:], sk[:, sl0])
                tg0_tiles = (xt0, {0: (cqt0, sqt0), 1: (cqt0, sqt0),
                                   2: (ckt0, skt0), 3: (ckt0, skt0)})
                nc.sync.dma_start(
                    wv_sb[:], wvT[:].rearrange("(k p) f -> p k f", p=128))

                for tg in range(NTG):
                    sl = slice(tg * 256, (tg + 1) * 256)
                    xt, cs_t = tg0_tiles if tg == 0 else load_tg(tg)
                    for fb in range(4):       # q0 q1 k0 k1
                        ps = ps1.tile([128, 256], F32, tag="ps1", name="ps")
                        for cc in range(NCC):
                            nc.tensor.matmul(
                                ps[:],
                                wqk_sb[:, cc, fb * 128:(fb + 1) * 128],
                                xt[:, cc, :],
                                start=(cc == 0), stop=(cc == NCC - 1))
                        qslice = qk_t[fb][:, sl]
                        nc.vector.tensor_copy(qslice, ps[:])
                        # RoPE on this 256-wide slice
                        pr = ps1.tile([D, 256], F32, tag="rot", name="pr",
                                      bufs=2)
                        nc.tensor.matmul(pr[:], prot_sb[:], qslice,
                                         start=True, stop=True)
                        ct, st_ = cs_t[fb]
                        t1 = xtp.tile([D, 256], F32, tag="t1", name="t1")
                        t2 = xtp.tile([D, 256], F32, tag="t2", name="t2")
                        nc.vector.tensor_mul(t1[:], pr[:], st_[:])
                        nc.vector.tensor_mul(t2[:], _f(qslice), ct[:])
                        nc.vector.tensor_add(qslice, t1[:], t2[:])
                    for tb in range(2):       # v natural
                        ps = ps1.tile([128, HLOC * D], F32, tag="ps1",
                                      name="ps")
                        for cc in range(NCC):
                            nc.tensor.matmul(
                                ps[:],
                                xt[:, cc, tb * 128:(tb + 1) * 128],
                                wv_sb[:, cc, :],
                                start=(cc == 0), stop=(cc == NCC - 1))
                        nc.scalar.copy(v_sb[:, tg * 2 + tb, :], ps[:])

            # ---------- phases 2+3 ----------
            with tc.tile_pool(name="aop", bufs=1) as aop:
                ao_t = [aop.tile([D, BT], F32R, tag=f"ao{h}", name=f"ao{h}")
                        for h in range(HLOC)]
                wo_sb = aop.tile([128, HLOC, C], F32R, tag="wo",
                                 name="wo_sb")

                # phase 2: attention
                with (
                    tc.tile_pool(name="att", bufs=3) as ap_,
                    tc.tile_pool(name="lp", bufs=2) as lp,
                    tc.tile_pool(name="ldram", bufs=2, space="DRAM") as ldp,
                    tc.tile_pool(name="pss", bufs=2, space="PSUM") as pss,
                    tc.tile_pool(name="pso", bufs=1, space="PSUM") as pso,
                ):
                    for h in range(HLOC):
                        if h == HLOC - 1:
                            nc.sync.dma_start(
                                wo_sb[:],
                                woT[:].rearrange("(h p) o -> p h o", p=128))
                        q_t, k_t = qk_t[h], qk_t[2 + h]
                        for tg2 in range(2):      # 1024-wide column groups
                            po = [pso.tile([D, 1024], F32, tag=f"po{b}",
                                           name=f"po{b}") for b in range(B)]
                            lacc = [lp.tile([128, 1024], F32R, tag=f"l{b}",
                                            name=f"l{b}") for b in range(B)]
                            for sc in range(NSC):
                                bt = ap_.tile([128, 1024], F32R, tag="bias",
                                              name="bt", bufs=4)
                                nc.sync.dma_start(
                                    bt[:],
                                    biasT[h, sc * 128:(sc + 1) * 128,
                                          tg2 * 1024:(tg2 + 1) * 1024])
                                for b in range(B):
                                    t0 = b * T + tg2 * 1024
                                    ps = pss.tile([128, 1024], F32, tag="ps",
                                                  name="ps")
                                    for hf in range(2):
                                        nc.tensor.matmul(
                                            ps[:, hf * 512:(hf + 1) * 512],
                                            ident_sb[:],
                                            bt[:, hf * 512:(hf + 1) * 512],
                                            start=True, stop=False,
                                            skip_group_check=True)
                                        nc.tensor.matmul(
                                            ps[:, hf * 512:(hf + 1) * 512],
                                            k_t[:, b * T + sc * 128:
                                                b * T + (sc + 1) * 128],
                                            q_t[:, t0 + hf * 512:
                                                t0 + (hf + 1) * 512],
                                            start=False, stop=True,
                                            skip_group_check=True)
                                    pe = ap_.tile([128, 1024], F32R, tag="pe",
                                                  name="pe", bufs=4)
                                    nc.scalar.activation(
                                        pe[:], ps[:],
                                        mybir.ActivationFunctionType.Exp)
                                    eng = nc.vector if b == 0 else nc.gpsimd
                                    if sc == 0:
                                        eng.tensor_copy(lacc[b][:], _f(pe[:]))
                                    else:
                                        eng.tensor_add(lacc[b][:],
                                                       _f(lacc[b][:]),
                                                       _f(pe[:]))
                                    for hf in range(2):
                                        nc.tensor.matmul(
                                            po[b][:, hf * 512:(hf + 1) * 512],
                                            v_sb[:, b * NSC + sc,
                                                 h * 128:(h + 1) * 128],
                                            pe[:, hf * 512:(hf + 1) * 512],
                                            start=(sc == 0),
                                            stop=(sc == NSC - 1),
                                            skip_group_check=True)
                            for b in range(B):
                                t0 = b * T + tg2 * 1024
                                ao_sl = ao_t[h][:, t0:t0 + 1024]
                                # evict unnormalized so po frees fast
                                nc.scalar.copy(ao_sl, po[b][:])
                                psl = pss.tile([1, 1024], F32, tag="ps",
                                               name="psl")
                                for hf in range(2):
                                    nc.tensor.matmul(
                                        psl[:, hf * 512:(hf + 1) * 512],
                                        ones_sb[:],
                                        lacc[b][:, hf * 512:(hf + 1) * 512],
                                        start=True, stop=True,
                                        skip_group_check=True)
                                linv = lacc[b][0:1, :]
                                with nc.allow_low_precision(
                                        reason="f32r bits == f32 bits"):
                                    nc.vector.reciprocal(linv, psl[:])
                                ldr = ldp.tile([1, 1024], F32R, tag="ldr",
                                               name="ldr")
                                nc.sync.dma_start(ldr[:], linv)
                                linb = ap_.tile([128, 1024], F32R, tag="pe",
                                                name="linb", bufs=4)
                                nc.sync.dma_start(
                                    linb[:], ldr[:].broadcast_to((128, 1024)))
                                nc.vector.tensor_mul(ao_sl, _f(ao_sl),
                                                     _f(linb[:]))

                    # phase 3: partial out-projection (same pool scope;
                    # psum via the po tags, no pool boundary barrier)
                    for ts in range(BT // 128):
                        r0 = ts * 128
                        for oh in range(2):
                            pt = pso.tile([D, 1024], F32, tag=f"po{oh}",
                                          name="pt")
                            for oc2 in range(2):
                                o0 = oh * 1024 + oc2 * 512
                                for hh in range(HLOC):
                                    nc.tensor.matmul(
                                        pt[:, oc2 * 512:(oc2 + 1) * 512],
                                        ao_t[hh][:, r0:r0 + 128],
                                        wo_sb[:, hh, o0:o0 + 512],
                                        start=(hh == 0),
                                        stop=(hh == HLOC - 1),
                                        skip_group_check=True)
                            stg = ap_.tile([128, 1024], F32, tag="stg",
                                           name="stg", bufs=3)
                            if (ts + oh) % 2 == 0:
                                nc.scalar.copy(stg[:], pt[:])
                            else:
                                nc.vector.tensor_copy(stg[:], pt[:])
                            nc.sync.dma_start(
                                out[r0:r0 + 128,
                                    oh * 1024:(oh + 1) * 1024],
                                stg[:])

    split_excess_waits(nc, limit=1)
    return nc


def prep_inputs(x, attn_mask, alibi_bias, Wqkv, Wout):
    """Host-side sharding: returns in_maps (list of 8 dicts)."""
    x = np.asarray(x, np.float32)
    attn_mask = np.asarray(attn_mask, np.float32)
    alibi_bias = np.asarray(alibi_bias, np.float32)
    Wqkv = np.asarray(Wqkv, np.float32)
    Wout = np.asarray(Wout, np.float32)

    xT = np.ascontiguousarray(x.reshape(BT, C).T)          # [C, BT]

    inv_freq = 1.0 / (ROPE_BASE ** (np.arange(0, D, 2, dtype=np.float32) / D))
    pos = np.arange(T, dtype=np.float32)
    freqs = np.einsum('i,j->ij', pos, inv_freq)
    emb = np.concatenate([freqs, freqs], axis=-1)          # [T, D]
    cosT = np.ascontiguousarray(np.cos(emb).T.astype(np.float32))  # [D, T]
    sinT = np.ascontiguousarray(np.sin(emb).T.astype(np.float32))
    cosT2 = np.concatenate([cosT, cosT], axis=1)           # [D, BT]
    sinT2 = np.concatenate([sinT, sinT], axis=1)
    cq = np.ascontiguousarray(cosT2 * SCALE)
    sq = np.ascontiguousarray(sinT2 * SCALE)
    ck = np.ascontiguousarray(cosT2)
    sk = np.ascontiguousarray(sinT2)

    P = np.zeros((D, D), np.float32)
    P[np.arange(64), np.arange(64) + 64] = -1.0
    P[np.arange(64) + 64, np.arange(64)] = 1.0
    protT = np.ascontiguousarray(P.T)

    Wq, Wk, Wv = Wqkv[0:C], Wqkv[C:2 * C], Wqkv[2 * C:3 * C]
    # bias per head, transposed: biasT_h[s, t] = mask[t, s] + alibi[h, t, s]
    biasT_all = np.ascontiguousarray(
        (attn_mask[None] + alibi_bias).transpose(0, 2, 1))

    in_maps = []
    for c in range(NCORES):
        lo, hi = c * HLOC * D, (c + 1) * HLOC * D
        qk_rows = np.concatenate([Wq[lo:hi], Wk[lo:hi]], axis=0)  # [512, C]
        in_maps.append({
            "xT": xT,
            "wqkT": np.ascontiguousarray(qk_rows.T),
            "wvT": np.ascontiguousarray(Wv[lo:hi].T),
            "prot": protT,
            "identw": np.eye(128, dtype=np.float32),
            "onesw": np.ones((128, 1), np.float32),
            "cq": cq, "sq": sq, "ck": ck, "sk": sk,
            "biasT": np.ascontiguousarray(biasT_all[c * HLOC:(c + 1) * HLOC]),
            "woT": np.ascontiguousarray(Wout[:, lo:hi].T),
        })
    return in_maps


# ---------------------------------------------------------------------------
# PJRT runner (adapted from concourse.bass2jax.run_bass_via_pjrt, without
# output-buffer donation so the jitted callable can be re-run for timing).
# ---------------------------------------------------------------------------
_CACHE = {}


def _get_runner():
    if "runner" in _CACHE:
        return _CACHE["runner"]

    import jax
    from jax.sharding import Mesh, PartitionSpec
    from jax.experimental.shard_map import shard_map
    from concourse.bass2jax import _bass_exec_p, install_neuronx_cc_hook

    install_neuronx_cc_hook()
    nc = build_bass()

    in_names, out_names, out_avals, zero_outs = [], [], [], []
    for alloc in nc.m.functions[0].allocations:
        if not isinstance(alloc, mybir.MemoryLocationSet):
            continue
        name = alloc.memorylocations[0].name
        if alloc.kind == "ExternalInput":
            in_names.append(name)
        elif alloc.kind == "ExternalOutput":
            out_names.append(name)
            shape = tuple(alloc.tensor_shape)
            dtype = mybir.dt.np(alloc.dtype)
            out_avals.append(jax.core.ShapedArray(shape, dtype))
            zero_outs.append(np.zeros(shape, dtype))
    n_params = len(in_names)
    all_names = in_names + out_names

    def _body(*args):
        outs = _bass_exec_p.bind(
            *args,
            out_avals=tuple(out_avals),
            in_names=tuple(all_names),
            out_names=tuple(out_names),
            lowering_input_output_aliases=(),
            sim_require_finite=True,
            sim_require_nnan=True,
            nc=nc,
        )
        return tuple(outs)

    devices = jax.devices()[:NCORES]
    mesh = Mesh(np.asarray(devices), ("core",))
    n_all = n_params + len(out_names)
    sharded = jax.jit(
        shard_map(
            _body, mesh=mesh,
            in_specs=(PartitionSpec("core"),) * n_all,
            out_specs=(PartitionSpec("core"),) * len(out_names),
            check_rep=False,
        ),
        keep_unused=True,
    )
    _CACHE["nc_obj"] = nc
    _CACHE["runner"] = (sharded, in_names, out_names, out_avals, zero_outs)
    return _CACHE["runner"]


def _run_device(in_maps):
    import jax
    sharded, in_names, out_names, out_avals, zero_outs = _get_runner()
    concat_in = [
        np.concatenate([in_maps[c][n] for c in range(NCORES)], axis=0)
        for n in in_names
    ]
    concat_zero = [
        np.zeros((NCORES * z.shape[0], *z.shape[1:]), z.dtype)
        for z in zero_outs
    ]
    args = [jax.device_put(a) for a in concat_in + concat_zero]
    _CACHE["last_args"] = args
    out_arrs = sharded(*args)
    out_arrs = [np.asarray(o) for o in out_arrs]
    return [
        {n: out_arrs[i].reshape(NCORES, *out_avals[i].shape)[c]
         for i, n in enumerate(out_names)}
        for c in range(NCORES)
    ]


def bench(n=10):
    """Re-run the cached jitted fn on the last inputs; returns per-call
    wall seconds. Includes dispatch/tunnel overhead."""
    import time as _time
    sharded = _CACHE["runner"][0]
    args = _CACHE["last_args"]
    times = []
    for _ in range(n):
        t0 = _time.perf_counter()
        res = sharded(*args)
        for r in res:
            r.block_until_ready()
        times.append(_time.perf_counter() - t0)
    return times


def kernel(x, attn_mask, alibi_bias, Wqkv, Wout):
    in_maps = prep_inputs(x, attn_mask, alibi_bias, Wqkv, Wout)
    results = _run_device(in_maps)
    acc = results[0]["out"].astype(np.float32).copy()
    for c in range(1, NCORES):
        acc += results[c]["out"]
    return acc.reshape(B, T, C)



def bench_async(ks=(1, 8, 16), n=4):
    """Queue k async dispatches of the cached jitted fn, block once.
    Marginal device time ~ (T(k2) - T(k1)) / (k2 - k1)."""
    import time as _time
    sharded = _CACHE["runner"][0]
    args = _CACHE["last_args"]
    out = {}
    for k in ks:
        best = float("inf")
        for _ in range(n):
            t0 = _time.perf_counter()
            rs = []
            for _i in range(k):
                rs.append(sharded(*args))
            for x in rs[-1]:
                x.block_until_ready()
            best = min(best, _time.perf_counter() - t0)
        out[k] = best
    return out


# revision 33
# speedup vs baseline: 1.0100x; 1.0004x over previous
"""Multi-head self-attention with ALiBi + RoPE, tensor-parallel over 8 NeuronCores.

Sharding: heads split across cores (2 heads/core). Each core computes its
heads' QKV projection, RoPE, attention (scores kept transposed [s, t] so no
PE transposes are needed), and a partial out-projection over its 256
channels. The 8 partial outputs are summed on the host.

Hardcoded problem shape: B=2, T=2048, C=2048, H=16, D=128.
"""

import sys

for _p in ('/opt/trn_rl_repo', '/root/.axon_site/_ro/trn_rl_repo'):
    if _p not in sys.path:
        sys.path.insert(0, _p)

import numpy as np

import bass_rust
import concourse.bass as bass
import concourse.tile as tile
import concourse.mybir as mybir

B, T, C, H = 2, 2048, 2048, 16
D = C // H            # 128
NCORES = 8
HLOC = H // NCORES    # heads per core = 2
ROPE_BASE = 10000.0
SCALE = 1.0 / np.sqrt(D)

F32 = mybir.dt.float32
F32R = mybir.dt.float32r
BT = B * T            # 4096 rows


def _r(ap):
    return ap.bitcast(F32R)


def _f(ap):
    return ap.bitcast(F32)


def split_excess_waits(nc, limit=1):
    """walrus CTRL codegen rejects >1 sem wait per instruction; move excess
    waits onto preceding NoOps on the same engine."""
    import copy as _copy
    ctrl_types = (bass_rust.InstDrain, bass_rust.InstNoOp, bass_rust.InstHalt,
                  bass_rust.InstEventSemaphore, bass_rust.InstAllEngineBarrier)
    ctr = 0
    for f in nc.m.functions:
        new_blocks = []
        for b in f.blocks:
            out = []
            changed = False
            for inst in b.instructions:
                si = inst.sync_info
                lim = limit
                if si is not None and si.on_wait and len(si.on_wait) > lim:
                    waits = list(si.on_wait)
                    excess, keep = waits[:-lim], waits[-lim:]
                    for i in range(0, len(excess), limit):
                        ctr += 1
                        nop = bass_rust.InstNoOp(
                            name=f"I-waitsplit-{ctr}", engine=inst.engine)
                        nop.sync_info = mybir.SyncInfo(
                            on_wait=excess[i:i + limit], on_update=[])
                        out.append(nop)
                    inst.sync_info = mybir.SyncInfo(
                        on_wait=keep, on_update=list(si.on_update or []))
                    changed = True
                out.append(inst)
            new_blocks.append(_copy.replace(b, instructions=out) if changed else b)
        f.blocks.clear()
        for nb in new_blocks:
            f.blocks.append(nb)
    return ctr


def build_bass():
    nc = bass.Bass(enable_partition_id=False)

    xT = nc.dram_tensor("xT", [C, BT], F32R, kind="ExternalInput")
    wqkT = nc.dram_tensor("wqkT", [C, 4 * D], F32R, kind="ExternalInput")
    wvT = nc.dram_tensor("wvT", [C, HLOC * D], F32R, kind="ExternalInput")
    prot = nc.dram_tensor("prot", [D, D], F32R, kind="ExternalInput")
    identw = nc.dram_tensor("identw", [128, 128], F32R, kind="ExternalInput")
    onesw = nc.dram_tensor("onesw", [128, 1], F32R, kind="ExternalInput")
    cq = nc.dram_tensor("cq", [D, BT], F32, kind="ExternalInput")
    sq = nc.dram_tensor("sq", [D, BT], F32, kind="ExternalInput")
    ck = nc.dram_tensor("ck", [D, BT], F32, kind="ExternalInput")
    sk = nc.dram_tensor("sk", [D, BT], F32, kind="ExternalInput")
    biasT = nc.dram_tensor("biasT", [HLOC, T, T], F32R, kind="ExternalInput")
    woT = nc.dram_tensor("woT", [HLOC * D, C], F32R, kind="ExternalInput")
    out = nc.dram_tensor("out", [BT, C], F32, kind="ExternalOutput")

    NCC = C // 128        # 16 contraction chunks
    NTG = BT // 256       # 16 t-groups in phase 1
    NSC = T // 128        # 16 s-chunks per batch

    with tile.TileContext(nc) as tc:
        with (
            tc.tile_pool(name="persist", bufs=1) as pp,
            tc.tile_pool(name="qkv", bufs=1) as qkvp,
        ):
            prot_sb = pp.tile([D, D], F32R, tag="prot", name="prot_sb")
            nc.sync.dma_start(prot_sb[:], prot[:])
            ones_sb = pp.tile([128, 1], F32R, tag="ones", name="ones_sb")
            nc.sync.dma_start(ones_sb[:], onesw[:])
            ident_sb = pp.tile([128, 128], F32R, tag="ident", name="ident_sb")
            nc.sync.dma_start(ident_sb[:], identw[:])

            # q0 q1 k0 k1 transposed [d, t]; v natural [t-in, chunk, f]
            qk_t = [qkvp.tile([D, BT], F32R, tag=f"qk{i}", name=f"qk{i}")
                    for i in range(4)]
            v_sb = qkvp.tile([128, BT // 128, HLOC * D], F32R, tag="v",
                             name="v_sb")

            # ---------- phase 1: QKV projection ----------
            with (
                tc.tile_pool(name="w1", bufs=1) as w1p,
                tc.tile_pool(name="xt", bufs=2) as xtp,
                tc.tile_pool(name="ps1", bufs=4, space="PSUM") as ps1,
            ):
                wqk_sb = w1p.tile([128, NCC, 4 * D], F32R, tag="wqk",
                                  name="wqk_sb")
                wv_sb = w1p.tile([128, NCC, HLOC * D], F32R, tag="wv",
                                 name="wv_sb")
                # chunk 0 of the weights first, then tg0's activations, so
                # the first matmul can start ~4us in; bulk loads after.
                nc.sync.dma_start(
                    wqk_sb[:, 0:1, :],
                    wqkT[0:128, :].rearrange("(k p) f -> p k f", p=128))
                nc.sync.dma_start(
                    wqk_sb[:, 1:4, :],
                    wqkT[128:512, :].rearrange("(k p) f -> p k f", p=128))

                def load_tg(tg):
                    sl = slice(tg * 256, (tg + 1) * 256)
                    xt = xtp.tile([128, NCC, 256], F32R, tag="xt", name="xt")
                    for xi in range(4):
                        nc.sync.dma_start(
                            xt[:, xi * 4:(xi + 1) * 4, :],
                            xT[xi * 512:(xi + 1) * 512, sl].rearrange(
                                "(k p) t -> p k t", p=128))
                    cqt = xtp.tile([D, 256], F32, tag="cqt", name="cqt")
                    sqt = xtp.tile([D, 256], F32, tag="sqt", name="sqt")
                    ckt = xtp.tile([D, 256], F32, tag="ckt", name="ckt")
                    skt = xtp.tile([D, 256], F32, tag="skt", name="skt")
                    nc.sync.dma_start(cqt[:], cq[:, sl])
                    nc.sync.dma_start(sqt[:], sq[:, sl])
                    nc.sync.dma_start(ckt[:], ck[:, sl])
                    nc.sync.dma_start(skt[:], sk[:, sl])
                    return xt, {0: (cqt, sqt), 1: (cqt, sqt),
                                2: (ckt, skt), 3: (ckt, skt)}

                # interleave remaining weight chunks with tg0 activations so
                # the fb0 accumulation is never starved mid-K.
                sl0 = slice(0, 256)
                xt0 = xtp.tile([128, NCC, 256], F32R, tag="xt", name="xt")
                for xi in range(4):
                    nc.sync.dma_start(
                        xt0[:, xi * 4:(xi + 1) * 4, :],
                        xT[xi * 512:(xi + 1) * 512, sl0].rearrange(
                            "(k p) t -> p k t", p=128))
                    if xi < 3:
                        nc.sync.dma_start(
                            wqk_sb[:, (xi + 1) * 4:(xi + 2) * 4, :],
                            wqkT[(xi + 1) * 512:(xi + 2) * 512, :].rearrange(
                                "(k p) f -> p k f", p=128))
                cqt0 = xtp.tile([D, 256], F32, tag="cqt", name="cqt")
                sqt0 = xtp.tile([D, 256], F32, tag="sqt", name="sqt")
                ckt0 = xtp.tile([D, 256], F32, tag="ckt", name="ckt")
                skt0 = xtp.tile([D, 256], F32, tag="skt", name="skt")
                nc.sync.dma_start(cqt0[:], cq[:, sl0])
                nc.sync.dma_start(sqt0[:], sq[:, sl0])
                nc.sync.dma_start(ckt0[:], ck[:, sl0])
                nc.sync.dma_start(skt0[:], sk[:, sl0])
                tg0_tiles = (xt0, {0: (cqt0, sqt0), 1: (cqt0, sqt0),
                                   2: (ckt0, skt0), 3: (ckt0, skt0)})
                nc.sync.dma_start(
                    wv_sb[:], wvT[:].rearrange("(k p) f -> p k f", p=128))

                for tg in range(NTG):
                    sl = slice(tg * 256, (tg + 1) * 256)
                    xt, cs_t = tg0_tiles if tg == 0 else load_tg(tg)
                    for fb in range(4):       # q0 q1 k0 k1
                        ps = ps1.tile([128, 256], F32, tag="ps1", name="ps")
                        for cc in range(NCC):
                            nc.tensor.matmul(
                                ps[:],
                                wqk_sb[:, cc, fb * 128:(fb + 1) * 128],
                                xt[:, cc, :],
                                start=(cc == 0), stop=(cc == NCC - 1))
                        qslice = qk_t[fb][:, sl]
                        nc.vector.tensor_copy(qslice, ps[:])
                        # RoPE on this 256-wide slice
                        pr = ps1.tile([D, 256], F32, tag="rot", name="pr",
                                      bufs=2)
                        nc.tensor.matmul(pr[:], prot_sb[:], qslice,
                                         start=True, stop=True)
                        ct, st_ = cs_t[fb]
                        t1 = xtp.tile([D, 256], F32, tag="t1", name="t1")
                        t2 = xtp.tile([D, 256], F32, tag="t2", name="t2")
                        nc.vector.tensor_mul(t1[:], pr[:], st_[:])
                        nc.vector.tensor_mul(t2[:], _f(qslice), ct[:])
                        nc.vector.tensor_add(qslice, t1[:], t2[:])
                    for tb in range(2):       # v natural
                        ps = ps1.tile([128, HLOC * D], F32, tag="ps1",
                                      name="ps")
                        for cc in range(NCC):
                            nc.tensor.matmul(
                                ps[:],
                                xt[:, cc, tb * 128:(tb + 1) * 128],
                                wv_sb[:, cc, :],
                                start=(cc == 0), stop=(cc == NCC - 1))
                        nc.scalar.copy(v_sb[:, tg * 2 + tb, :], ps[:])

            # ---------- phases 2+3 ----------
            with tc.tile_pool(name="aop", bufs=1) as aop:
                ao_t = [aop.tile([D, BT], F32R, tag=f"ao{h}", name=f"ao{h}")
                        for h in range(HLOC)]
                wo_sb = aop.tile([128, HLOC, C], F32R, tag="wo",
                                 name="wo_sb")

                # phase 2: attention
                with (
                    tc.tile_pool(name="att", bufs=3) as ap_,
                    tc.tile_pool(name="lp", bufs=2) as lp,
                    tc.tile_pool(name="ldram", bufs=2, space="DRAM") as ldp,
                    tc.tile_pool(name="pss", bufs=2, space="PSUM") as pss,
                    tc.tile_pool(name="pso", bufs=1, space="PSUM") as pso,
                ):
                    for h in range(HLOC):
                        if h == HLOC - 1:
                            nc.sync.dma_start(
                                wo_sb[:],
                                woT[:].rearrange("(h p) o -> p h o", p=128))
                        q_t, k_t = qk_t[h], qk_t[2 + h]
                        for tg2 in range(2):      # 1024-wide column groups
                            po = [pso.tile([D, 1024], F32, tag=f"po{b}",
                                           name=f"po{b}") for b in range(B)]
                            lacc = [lp.tile([128, 1024], F32R, tag=f"l{b}",
                                            name=f"l{b}") for b in range(B)]
                            for sc in range(NSC):
                                bt = ap_.tile([128, 1024], F32R, tag="bias",
                                              name="bt", bufs=4)
                                nc.sync.dma_start(
                                    bt[:],
                                    biasT[h, sc * 128:(sc + 1) * 128,
                                          tg2 * 1024:(tg2 + 1) * 1024])
                                for b in range(B):
                                    t0 = b * T + tg2 * 1024
                                    ps = pss.tile([128, 1024], F32, tag="ps",
                                                  name="ps")
                                    for hf in range(2):
                                        nc.tensor.matmul(
                                            ps[:, hf * 512:(hf + 1) * 512],
                                            ident_sb[:],
                                            bt[:, hf * 512:(hf + 1) * 512],
                                            start=True, stop=False,
                                            skip_group_check=True)
                                        nc.tensor.matmul(
                                            ps[:, hf * 512:(hf + 1) * 512],
                                            k_t[:, b * T + sc * 128:
                                                b * T + (sc + 1) * 128],
                                            q_t[:, t0 + hf * 512:
                                                t0 + (hf + 1) * 512],
                                            start=False, stop=True,
                                            skip_group_check=True)
                                    pe = ap_.tile([128, 1024], F32R, tag="pe",
                                                  name="pe", bufs=4)
                                    nc.scalar.activation(
                                        pe[:], ps[:],
                                        mybir.ActivationFunctionType.Exp)
                                    eng = nc.vector if b == 0 else nc.gpsimd
                                    if sc == 0:
                                        eng.tensor_copy(lacc[b][:], _f(pe[:]))
                                    else:
                                        eng.tensor_add(lacc[b][:],
                                                       _f(lacc[b][:]),
                                                       _f(pe[:]))
                                    for hf in range(2):
                                        nc.tensor.matmul(
                                            po[b][:, hf * 512:(hf + 1) * 512],
                                            v_sb[:, b * NSC + sc,
                                                 h * 128:(h + 1) * 128],
                                            pe[:, hf * 512:(hf + 1) * 512],
                                            start=(sc == 0),
                                            stop=(sc == NSC - 1),
                                            skip_group_check=True)
                            for b in range(B):
                                t0 = b * T + tg2 * 1024
                                ao_sl = ao_t[h][:, t0:t0 + 1024]
                                # evict unnormalized so po frees fast
                                nc.scalar.copy(ao_sl, po[b][:])
                                psl = pss.tile([1, 1024], F32, tag="ps",
                                               name="psl")
                                for hf in range(2):
                                    nc.tensor.matmul(
                                        psl[:, hf * 512:(hf + 1) * 512],
                                        ones_sb[:],
                                        lacc[b][:, hf * 512:(hf + 1) * 512],
                                        start=True, stop=True,
                                        skip_group_check=True)
                                linv = lacc[b][0:1, :]
                                with nc.allow_low_precision(
                                        reason="f32r bits == f32 bits"):
                                    nc.vector.reciprocal(linv, psl[:])
                                ldr = ldp.tile([1, 1024], F32R, tag="ldr",
                                               name="ldr")
                                nc.sync.dma_start(ldr[:], linv)
                                linb = ap_.tile([128, 1024], F32R, tag="pe",
                                                name="linb", bufs=4)
                                nc.sync.dma_start(
                                    linb[:], ldr[:].broadcast_to((128, 1024)))
                                nc.vector.tensor_mul(ao_sl, _f(ao_sl),
                                                     _f(linb[:]))

                    # phase 3: partial out-projection (same pool scope;
                    # psum via the po tags, no pool boundary barrier)
                    for ts in range(BT // 128):
                        r0 = ts * 128
                        for oh in range(2):
                            pt = pso.tile([D, 1024], F32, tag=f"po{oh}",
                                          name="pt")
                            for oc2 in range(2):
                                o0 = oh * 1024 + oc2 * 512
                                for hh in range(HLOC):
                                    nc.tensor.matmul(
                                        pt[:, oc2 * 512:(oc2 + 1) * 512],
                                        ao_t[hh][:, r0:r0 + 128],
                                        wo_sb[:, hh, o0:o0 + 512],
                                        start=(hh == 0),
                                        stop=(hh == HLOC - 1),
                                        skip_group_check=True)
                            stg = ap_.tile([128, 1024], F32, tag="stg",
                                           name="stg", bufs=3)
                            if (ts + oh) % 2 == 0:
                                nc.scalar.copy(stg[:], pt[:])
                            else:
                                nc.vector.tensor_copy(stg[:], pt[:])
                            nc.sync.dma_start(
                                out[r0:r0 + 128,
                                    oh * 1024:(oh + 1) * 1024],
                                stg[:])

    split_excess_waits(nc, limit=1)
    return nc


def prep_inputs(x, attn_mask, alibi_bias, Wqkv, Wout):
    """Host-side sharding: returns in_maps (list of 8 dicts)."""
    x = np.asarray(x, np.float32)
    attn_mask = np.asarray(attn_mask, np.float32)
    alibi_bias = np.asarray(alibi_bias, np.float32)
    Wqkv = np.asarray(Wqkv, np.float32)
    Wout = np.asarray(Wout, np.float32)

    xT = np.ascontiguousarray(x.reshape(BT, C).T)          # [C, BT]

    inv_freq = 1.0 / (ROPE_BASE ** (np.arange(0, D, 2, dtype=np.float32) / D))
    pos = np.arange(T, dtype=np.float32)
    freqs = np.einsum('i,j->ij', pos, inv_freq)
    emb = np.concatenate([freqs, freqs], axis=-1)          # [T, D]
    cosT = np.ascontiguousarray(np.cos(emb).T.astype(np.float32))  # [D, T]
    sinT = np.ascontiguousarray(np.sin(emb).T.astype(np.float32))
    cosT2 = np.concatenate([cosT, cosT], axis=1)           # [D, BT]
    sinT2 = np.concatenate([sinT, sinT], axis=1)
    cq = np.ascontiguousarray(cosT2 * SCALE)
    sq = np.ascontiguousarray(sinT2 * SCALE)
    ck = np.ascontiguousarray(cosT2)
    sk = np.ascontiguousarray(sinT2)

    P = np.zeros((D, D), np.float32)
    P[np.arange(64), np.arange(64) + 64] = -1.0
    P[np.arange(64) + 64, np.arange(64)] = 1.0
    protT = np.ascontiguousarray(P.T)

    Wq, Wk, Wv = Wqkv[0:C], Wqkv[C:2 * C], Wqkv[2 * C:3 * C]
    # bias per head, transposed: biasT_h[s, t] = mask[t, s] + alibi[h, t, s]
    biasT_all = np.ascontiguousarray(
        (attn_mask[None] + alibi_bias).transpose(0, 2, 1))

    in_maps = []
    for c in range(NCORES):
        lo, hi = c * HLOC * D, (c + 1) * HLOC * D
        qk_rows = np.concatenate([Wq[lo:hi], Wk[lo:hi]], axis=0)  # [512, C]
        in_maps.append({
            "xT": xT,
            "wqkT": np.ascontiguousarray(qk_rows.T),
            "wvT": np.ascontiguousarray(Wv[lo:hi].T),
            "prot": protT,
            "identw": np.eye(128, dtype=np.float32),
            "onesw": np.ones((128, 1), np.float32),
            "cq": cq, "sq": sq, "ck": ck, "sk": sk,
            "biasT": np.ascontiguousarray(biasT_all[c * HLOC:(c + 1) * HLOC]),
            "woT": np.ascontiguousarray(Wout[:, lo:hi].T),
        })
    return in_maps


# ---------------------------------------------------------------------------
# PJRT runner (adapted from concourse.bass2jax.run_bass_via_pjrt, without
# output-buffer donation so the jitted callable can be re-run for timing).
# ---------------------------------------------------------------------------
_CACHE = {}


def _get_runner():
    if "runner" in _CACHE:
        return _CACHE["runner"]

    import jax
    from jax.sharding import Mesh, PartitionSpec
    from jax.experimental.shard_map import shard_map
    from concourse.bass2jax import _bass_exec_p, install_neuronx_cc_hook

    install_neuronx_cc_hook()
    nc = build_bass()

    in_names, out_names, out_avals, zero_outs = [], [], [], []
    for alloc in nc.m.functions[0].allocations:
        if not isinstance(alloc, mybir.MemoryLocationSet):
            continue
        name = alloc.memorylocations[0].name
        if alloc.kind == "ExternalInput":
            in_names.append(name)
        elif alloc.kind == "ExternalOutput":
            out_names.append(name)
            shape = tuple(alloc.tensor_shape)
            dtype = mybir.dt.np(alloc.dtype)
            out_avals.append(jax.core.ShapedArray(shape, dtype))
            zero_outs.append(np.zeros(shape, dtype))
    n_params = len(in_names)
    all_names = in_names + out_names

    def _body(*args):
        outs = _bass_exec_p.bind(
            *args,
            out_avals=tuple(out_avals),
            in_names=tuple(all_names),
            out_names=tuple(out_names),
            lowering_input_output_aliases=(),
            sim_require_finite=True,
            sim_require_nnan=True,
            nc=nc,
        )
        return tuple(outs)

    devices = jax.devices()[:NCORES]
    mesh = Mesh(np.asarray(devices), ("core",))
    n_all = n_params + len(out_names)
    sharded = jax.jit(
        shard_map(
            _body, mesh=mesh,
            in_specs=(PartitionSpec("core"),) * n_all,
            out_specs=(PartitionSpec("core"),) * len(out_names),
            check_rep=False,
        ),
        keep_unused=True,
    )
    _CACHE["nc_obj"] = nc
    _CACHE["runner"] = (sharded, in_names, out_names, out_avals, zero_outs)
    return _CACHE["runner"]


def _run_device(in_maps):
    import jax
    sharded, in_names, out_names, out_avals, zero_outs = _get_runner()
    concat_in = [
        np.concatenate([in_maps[c][n] for c in range(NCORES)], axis=0)
        for n in in_names
    ]
    concat_zero = [
        np.zeros((NCORES * z.shape[0], *z.shape[1:]), z.dtype)
        for z in zero_outs
    ]
    args = [jax.device_put(a) for a in concat_in + concat_zero]
    _CACHE["last_args"] = args
    out_arrs = sharded(*args)
    out_arrs = [np.asarray(o) for o in out_arrs]
    return [
        {n: out_arrs[i].reshape(NCORES, *out_avals[i].shape)[c]
         for i, n in enumerate(out_names)}
        for c in range(NCORES)
    ]


def bench(n=10):
    """Re-run the cached jitted fn on the last inputs; returns per-call
    wall seconds. Includes dispatch/tunnel overhead."""
    import time as _time
    sharded = _CACHE["runner"][0]
    args = _CACHE["last_args"]
    times = []
    for _ in range(n):
        t0 = _time.perf_counter()
        res = sharded(*args)
        for r in res:
            r.block_until_ready()
        times.append(_time.perf_counter() - t0)
    return times


def kernel(x, attn_mask, alibi_bias, Wqkv, Wout):
    in_maps = prep_inputs(x, attn_mask, alibi_bias, Wqkv, Wout)
    results = _run_device(in_maps)
    acc = results[0]["out"].astype(np.float32).copy()
    for c in range(1, NCORES):
        acc += results[c]["out"]
    return acc.reshape(B, T, C)



def bench_async(ks=(1, 8, 16), n=4):
    """Queue k async dispatches of the cached jitted fn, block once.
    Marginal device time ~ (T(k2) - T(k1)) / (k2 - k1)."""
    import time as _time
    sharded = _CACHE["runner"][0]
    args = _CACHE["last_args"]
    out = {}
    for k in ks:
        best = float("inf")
        for _ in range(n):
            t0 = _time.perf_counter()
            rs = []
            for _i in range(k):
                rs.append(sharded(*args))
            for x in rs[-1]:
                x.block_until_ready()
            best = min(best, _time.perf_counter() - t0)
        out[k] = best
    return out


# revision 36
# speedup vs baseline: 1.0361x; 1.0258x over previous
"""Multi-head self-attention with ALiBi + RoPE, tensor-parallel over 8 NeuronCores.

Sharding: heads split across cores (2 heads/core). Each core computes its
heads' QKV projection, RoPE, attention (scores kept transposed [s, t] so no
PE transposes are needed), and a partial out-projection over its 256
channels. The 8 partial outputs are summed on the host.

Hardcoded problem shape: B=2, T=2048, C=2048, H=16, D=128.
"""

import sys

for _p in ('/opt/trn_rl_repo', '/root/.axon_site/_ro/trn_rl_repo'):
    if _p not in sys.path:
        sys.path.insert(0, _p)

import numpy as np

import bass_rust
import concourse.bass as bass
import concourse.tile as tile
import concourse.mybir as mybir

B, T, C, H = 2, 2048, 2048, 16
D = C // H            # 128
NCORES = 8
HLOC = H // NCORES    # heads per core = 2
ROPE_BASE = 10000.0
SCALE = 1.0 / np.sqrt(D)

F32 = mybir.dt.float32
F32R = mybir.dt.float32r
BT = B * T            # 4096 rows


def _r(ap):
    return ap.bitcast(F32R)


def _f(ap):
    return ap.bitcast(F32)


def split_excess_waits(nc, limit=1):
    """walrus CTRL codegen rejects >1 sem wait per instruction; move excess
    waits onto preceding NoOps on the same engine."""
    import copy as _copy
    ctrl_types = (bass_rust.InstDrain, bass_rust.InstNoOp, bass_rust.InstHalt,
                  bass_rust.InstEventSemaphore, bass_rust.InstAllEngineBarrier)
    ctr = 0
    for f in nc.m.functions:
        new_blocks = []
        for b in f.blocks:
            out = []
            changed = False
            for inst in b.instructions:
                si = inst.sync_info
                lim = limit
                if si is not None and si.on_wait and len(si.on_wait) > lim:
                    waits = list(si.on_wait)
                    excess, keep = waits[:-lim], waits[-lim:]
                    for i in range(0, len(excess), limit):
                        ctr += 1
                        nop = bass_rust.InstNoOp(
                            name=f"I-waitsplit-{ctr}", engine=inst.engine)
                        nop.sync_info = mybir.SyncInfo(
                            on_wait=excess[i:i + limit], on_update=[])
                        out.append(nop)
                    inst.sync_info = mybir.SyncInfo(
                        on_wait=keep, on_update=list(si.on_update or []))
                    changed = True
                out.append(inst)
            new_blocks.append(_copy.replace(b, instructions=out) if changed else b)
        f.blocks.clear()
        for nb in new_blocks:
            f.blocks.append(nb)
    return ctr


def build_bass():
    nc = bass.Bass(enable_partition_id=False)

    xT = nc.dram_tensor("xT", [C, BT], F32R, kind="ExternalInput")
    wqkT = nc.dram_tensor("wqkT", [C, 4 * D], F32R, kind="ExternalInput")
    wvT = nc.dram_tensor("wvT", [C, HLOC * D], F32R, kind="ExternalInput")
    prot = nc.dram_tensor("prot", [D, D], F32R, kind="ExternalInput")
    identw = nc.dram_tensor("identw", [128, 128], F32R, kind="ExternalInput")
    onesw = nc.dram_tensor("onesw", [128, 1], F32R, kind="ExternalInput")
    cq = nc.dram_tensor("cq", [D, BT], F32, kind="ExternalInput")
    sq = nc.dram_tensor("sq", [D, BT], F32, kind="ExternalInput")
    ck = nc.dram_tensor("ck", [D, BT], F32, kind="ExternalInput")
    sk = nc.dram_tensor("sk", [D, BT], F32, kind="ExternalInput")
    biasT = nc.dram_tensor("biasT", [HLOC, T, T], F32R, kind="ExternalInput")
    woT = nc.dram_tensor("woT", [HLOC * D, C], F32R, kind="ExternalInput")
    out = nc.dram_tensor("out", [BT, C], F32, kind="ExternalOutput")

    NCC = C // 128        # 16 contraction chunks
    NTG = BT // 256       # 16 t-groups in phase 1
    NSC = T // 128        # 16 s-chunks per batch

    with tile.TileContext(nc) as tc:
        with (
            tc.tile_pool(name="persist", bufs=1) as pp,
            tc.tile_pool(name="qkv", bufs=1) as qkvp,
        ):
            prot_sb = pp.tile([D, D], F32R, tag="prot", name="prot_sb")
            nc.sync.dma_start(prot_sb[:], prot[:])
            ones_sb = pp.tile([128, 1], F32R, tag="ones", name="ones_sb")
            nc.sync.dma_start(ones_sb[:], onesw[:])
            ident_sb = pp.tile([128, 128], F32R, tag="ident", name="ident_sb")
            nc.sync.dma_start(ident_sb[:], identw[:])

            # q0 q1 k0 k1 transposed [d, t]; v natural [t-in, chunk, f]
            qk_t = [qkvp.tile([D, BT], F32R, tag=f"qk{i}", name=f"qk{i}")
                    for i in range(4)]
            v_sb = qkvp.tile([128, BT // 128, HLOC * D], F32R, tag="v",
                             name="v_sb")

            # ---------- phase 1: QKV projection ----------
            with (
                tc.tile_pool(name="w1", bufs=1) as w1p,
                tc.tile_pool(name="xt", bufs=2) as xtp,
                tc.tile_pool(name="ps1", bufs=4, space="PSUM") as ps1,
            ):
                wqk_sb = w1p.tile([128, NCC, 4 * D], F32R, tag="wqk",
                                  name="wqk_sb")
                wv_sb = w1p.tile([128, NCC, HLOC * D], F32R, tag="wv",
                                 name="wv_sb")
                # chunk 0 of the weights first, then tg0's activations, so
                # the first matmul can start ~4us in; bulk loads after.
                nc.sync.dma_start(
                    wqk_sb[:, 0:1, :],
                    wqkT[0:128, :].rearrange("(k p) f -> p k f", p=128))
                nc.sync.dma_start(
                    wqk_sb[:, 1:4, :],
                    wqkT[128:512, :].rearrange("(k p) f -> p k f", p=128))

                def load_tg(tg):
                    sl = slice(tg * 256, (tg + 1) * 256)
                    xt = xtp.tile([128, NCC, 256], F32R, tag="xt", name="xt")
                    for xi in range(4):
                        nc.sync.dma_start(
                            xt[:, xi * 4:(xi + 1) * 4, :],
                            xT[xi * 512:(xi + 1) * 512, sl].rearrange(
                                "(k p) t -> p k t", p=128))
                    cqt = xtp.tile([D, 256], F32, tag="cqt", name="cqt")
                    sqt = xtp.tile([D, 256], F32, tag="sqt", name="sqt")
                    ckt = xtp.tile([D, 256], F32, tag="ckt", name="ckt")
                    skt = xtp.tile([D, 256], F32, tag="skt", name="skt")
                    nc.sync.dma_start(cqt[:], cq[:, sl])
                    nc.sync.dma_start(sqt[:], sq[:, sl])
                    nc.sync.dma_start(ckt[:], ck[:, sl])
                    nc.sync.dma_start(skt[:], sk[:, sl])
                    return xt, {0: (cqt, sqt), 1: (cqt, sqt),
                                2: (ckt, skt), 3: (ckt, skt)}

                # interleave remaining weight chunks with tg0 activations so
                # the fb0 accumulation is never starved mid-K.
                sl0 = slice(0, 256)
                xt0 = xtp.tile([128, NCC, 256], F32R, tag="xt", name="xt")
                for xi in range(4):
                    nc.sync.dma_start(
                        xt0[:, xi * 4:(xi + 1) * 4, :],
                        xT[xi * 512:(xi + 1) * 512, sl0].rearrange(
                            "(k p) t -> p k t", p=128))
                    if xi < 3:
                        nc.sync.dma_start(
                            wqk_sb[:, (xi + 1) * 4:(xi + 2) * 4, :],
                            wqkT[(xi + 1) * 512:(xi + 2) * 512, :].rearrange(
                                "(k p) f -> p k f", p=128))
                cqt0 = xtp.tile([D, 256], F32, tag="cqt", name="cqt")
                sqt0 = xtp.tile([D, 256], F32, tag="sqt", name="sqt")
                ckt0 = xtp.tile([D, 256], F32, tag="ckt", name="ckt")
                skt0 = xtp.tile([D, 256], F32, tag="skt", name="skt")
                nc.sync.dma_start(cqt0[:], cq[:, sl0])
                nc.sync.dma_start(sqt0[:], sq[:, sl0])
                nc.sync.dma_start(ckt0[:], ck[:, sl0])
                nc.sync.dma_start(skt0[:], sk[:, sl0])
                tg0_tiles = (xt0, {0: (cqt0, sqt0), 1: (cqt0, sqt0),
                                   2: (ckt0, skt0), 3: (ckt0, skt0)})
                nc.sync.dma_start(
                    wv_sb[:], wvT[:].rearrange("(k p) f -> p k f", p=128))

                for tg in range(NTG):
                    sl = slice(tg * 256, (tg + 1) * 256)
                    xt, cs_t = tg0_tiles if tg == 0 else load_tg(tg)
                    for fb in range(4):       # q0 q1 k0 k1
                        ps = ps1.tile([128, 256], F32, tag="ps1", name="ps")
                        for cc in range(NCC):
                            nc.tensor.matmul(
                                ps[:],
                                wqk_sb[:, cc, fb * 128:(fb + 1) * 128],
                                xt[:, cc, :],
                                start=(cc == 0), stop=(cc == NCC - 1))
                        qslice = qk_t[fb][:, sl]
                        nc.vector.tensor_copy(qslice, ps[:])
                        # RoPE on this 256-wide slice
                        pr = ps1.tile([D, 256], F32, tag="rot", name="pr",
                                      bufs=2)
                        nc.tensor.matmul(pr[:], prot_sb[:], qslice,
                                         start=True, stop=True)
                        ct, st_ = cs_t[fb]
                        t1 = xtp.tile([D, 256], F32, tag="t1", name="t1")
                        t2 = xtp.tile([D, 256], F32, tag="t2", name="t2")
                        nc.vector.tensor_mul(t1[:], pr[:], st_[:])
                        nc.vector.tensor_mul(t2[:], _f(qslice), ct[:])
                        nc.vector.tensor_add(qslice, t1[:], t2[:])
                    for tb in range(2):       # v natural
                        ps = ps1.tile([128, HLOC * D], F32, tag="ps1",
                                      name="ps")
                        for cc in range(NCC):
                            nc.tensor.matmul(
                                ps[:],
                                xt[:, cc, tb * 128:(tb + 1) * 128],
                                wv_sb[:, cc, :],
                                start=(cc == 0), stop=(cc == NCC - 1))
                        nc.scalar.copy(v_sb[:, tg * 2 + tb, :], ps[:])

            # ---------- phases 2+3 ----------
            with tc.tile_pool(name="aop", bufs=1) as aop:
                ao_t = [aop.tile([D, BT], F32R, tag=f"ao{h}", name=f"ao{h}")
                        for h in range(HLOC)]
                wo_sb = aop.tile([128, HLOC, C], F32R, tag="wo",
                                 name="wo_sb")

                # phase 2: attention
                with (
                    tc.tile_pool(name="att", bufs=3) as ap_,
                    tc.tile_pool(name="lp", bufs=2) as lp,
                    tc.tile_pool(name="ldram", bufs=2, space="DRAM") as ldp,
                    tc.tile_pool(name="pss", bufs=2, space="PSUM") as pss,
                    tc.tile_pool(name="pso", bufs=1, space="PSUM") as pso,
                ):
                    for h in range(HLOC):
                        if h == HLOC - 1:
                            nc.sync.dma_start(
                                wo_sb[:],
                                woT[:].rearrange("(h p) o -> p h o", p=128))
                        q_t, k_t = qk_t[h], qk_t[2 + h]
                        for tg2 in range(2):      # 1024-wide column groups
                            po = [pso.tile([D, 1024], F32, tag=f"po{b}",
                                           name=f"po{b}") for b in range(B)]
                            lacc = [lp.tile([128, 1024], F32R, tag=f"l{b}",
                                            name=f"l{b}") for b in range(B)]
                            for sc in range(NSC):
                                bt = ap_.tile([128, 1024], F32R, tag="bias",
                                              name="bt", bufs=3)
                                nc.sync.dma_start(
                                    bt[:],
                                    biasT[h, sc * 128:(sc + 1) * 128,
                                          tg2 * 1024:(tg2 + 1) * 1024])
                                for b in range(B):
                                    t0 = b * T + tg2 * 1024
                                    ps = pss.tile([128, 1024], F32, tag="ps",
                                                  name="ps")
                                    for hf in range(2):
                                        nc.tensor.matmul(
                                            ps[:, hf * 512:(hf + 1) * 512],
                                            ident_sb[:],
                                            bt[:, hf * 512:(hf + 1) * 512],
                                            start=True, stop=False,
                                            skip_group_check=True)
                                        nc.tensor.matmul(
                                            ps[:, hf * 512:(hf + 1) * 512],
                                            k_t[:, b * T + sc * 128:
                                                b * T + (sc + 1) * 128],
                                            q_t[:, t0 + hf * 512:
                                                t0 + (hf + 1) * 512],
                                            start=False, stop=True,
                                            skip_group_check=True)
                                    pe = ap_.tile([128, 1024], F32R, tag="pe",
                                                  name="pe", bufs=4)
                                    nc.scalar.activation(
                                        pe[:], ps[:],
                                        mybir.ActivationFunctionType.Exp)
                                    eng = nc.vector if b == 0 else nc.gpsimd
                                    if sc == 0:
                                        eng.tensor_copy(lacc[b][:], _f(pe[:]))
                                    else:
                                        eng.tensor_add(lacc[b][:],
                                                       _f(lacc[b][:]),
                                                       _f(pe[:]))
                                    for hf in range(2):
                                        nc.tensor.matmul(
                                            po[b][:, hf * 512:(hf + 1) * 512],
                                            v_sb[:, b * NSC + sc,
                                                 h * 128:(h + 1) * 128],
                                            pe[:, hf * 512:(hf + 1) * 512],
                                            start=(sc == 0),
                                            stop=(sc == NSC - 1),
                                            skip_group_check=True)
                            for b in range(B):
                                t0 = b * T + tg2 * 1024
                                ao_sl = ao_t[h][:, t0:t0 + 1024]
                                # evict unnormalized so po frees fast
                                nc.scalar.copy(ao_sl, po[b][:])
                                psl = pss.tile([1, 1024], F32, tag="ps",
                                               name="psl")
                                for hf in range(2):
                                    nc.tensor.matmul(
                                        psl[:, hf * 512:(hf + 1) * 512],
                                        ones_sb[:],
                                        lacc[b][:, hf * 512:(hf + 1) * 512],
                                        start=True, stop=True,
                                        skip_group_check=True)
                                linv = lacc[b][0:1, :]
                                with nc.allow_low_precision(
                                        reason="f32r bits == f32 bits"):
                                    nc.vector.reciprocal(linv, psl[:])
                                ldr = ldp.tile([1, 1024], F32R, tag="ldr",
                                               name="ldr")
                                nc.sync.dma_start(ldr[:], linv)
                                linb = ap_.tile([128, 1024], F32R, tag="pe",
                                                name="linb", bufs=4)
                                nc.sync.dma_start(
                                    linb[:], ldr[:].broadcast_to((128, 1024)))
                                nc.vector.tensor_mul(ao_sl, _f(ao_sl),
                                                     _f(linb[:]))

                    # phase 3: partial out-projection (same pool scope;
                    # psum via the po tags, no pool boundary barrier)
                    for ts in range(BT // 128):
                        r0 = ts * 128
                        for oh in range(2):
                            pt = pso.tile([D, 1024], F32, tag=f"po{oh}",
                                          name="pt")
                            for oc2 in range(2):
                                o0 = oh * 1024 + oc2 * 512
                                for hh in range(HLOC):
                                    nc.tensor.matmul(
                                        pt[:, oc2 * 512:(oc2 + 1) * 512],
                                        ao_t[hh][:, r0:r0 + 128],
                                        wo_sb[:, hh, o0:o0 + 512],
                                        start=(hh == 0),
                                        stop=(hh == HLOC - 1),
                                        skip_group_check=True)
                            stg = ap_.tile([128, 1024], F32, tag="stg",
                                           name="stg", bufs=4)
                            if (ts + oh) % 2 == 0:
                                nc.scalar.copy(stg[:], pt[:])
                            else:
                                nc.vector.tensor_copy(stg[:], pt[:])
                            nc.sync.dma_start(
                                out[r0:r0 + 128,
                                    oh * 1024:(oh + 1) * 1024],
                                stg[:])

    split_excess_waits(nc, limit=1)
    return nc


def prep_inputs(x, attn_mask, alibi_bias, Wqkv, Wout):
    """Host-side sharding: returns in_maps (list of 8 dicts)."""
    x = np.asarray(x, np.float32)
    attn_mask = np.asarray(attn_mask, np.float32)
    alibi_bias = np.asarray(alibi_bias, np.float32)
    Wqkv = np.asarray(Wqkv, np.float32)
    Wout = np.asarray(Wout, np.float32)

    xT = np.ascontiguousarray(x.reshape(BT, C).T)          # [C, BT]

    inv_freq = 1.0 / (ROPE_BASE ** (np.arange(0, D, 2, dtype=np.float32) / D))
    pos = np.arange(T, dtype=np.float32)
    freqs = np.einsum('i,j->ij', pos, inv_freq)
    emb = np.concatenate([freqs, freqs], axis=-1)          # [T, D]
    cosT = np.ascontiguousarray(np.cos(emb).T.astype(np.float32))  # [D, T]
    sinT = np.ascontiguousarray(np.sin(emb).T.astype(np.float32))
    cosT2 = np.concatenate([cosT, cosT], axis=1)           # [D, BT]
    sinT2 = np.concatenate([sinT, sinT], axis=1)
    cq = np.ascontiguousarray(cosT2 * SCALE)
    sq = np.ascontiguousarray(sinT2 * SCALE)
    ck = np.ascontiguousarray(cosT2)
    sk = np.ascontiguousarray(sinT2)

    P = np.zeros((D, D), np.float32)
    P[np.arange(64), np.arange(64) + 64] = -1.0
    P[np.arange(64) + 64, np.arange(64)] = 1.0
    protT = np.ascontiguousarray(P.T)

    Wq, Wk, Wv = Wqkv[0:C], Wqkv[C:2 * C], Wqkv[2 * C:3 * C]
    # bias per head, transposed: biasT_h[s, t] = mask[t, s] + alibi[h, t, s]
    biasT_all = np.ascontiguousarray(
        (attn_mask[None] + alibi_bias).transpose(0, 2, 1))

    in_maps = []
    for c in range(NCORES):
        lo, hi = c * HLOC * D, (c + 1) * HLOC * D
        qk_rows = np.concatenate([Wq[lo:hi], Wk[lo:hi]], axis=0)  # [512, C]
        in_maps.append({
            "xT": xT,
            "wqkT": np.ascontiguousarray(qk_rows.T),
            "wvT": np.ascontiguousarray(Wv[lo:hi].T),
            "prot": protT,
            "identw": np.eye(128, dtype=np.float32),
            "onesw": np.ones((128, 1), np.float32),
            "cq": cq, "sq": sq, "ck": ck, "sk": sk,
            "biasT": np.ascontiguousarray(biasT_all[c * HLOC:(c + 1) * HLOC]),
            "woT": np.ascontiguousarray(Wout[:, lo:hi].T),
        })
    return in_maps


# ---------------------------------------------------------------------------
# PJRT runner (adapted from concourse.bass2jax.run_bass_via_pjrt, without
# output-buffer donation so the jitted callable can be re-run for timing).
# ---------------------------------------------------------------------------
_CACHE = {}


def _get_runner():
    if "runner" in _CACHE:
        return _CACHE["runner"]

    import jax
    from jax.sharding import Mesh, PartitionSpec
    from jax.experimental.shard_map import shard_map
    from concourse.bass2jax import _bass_exec_p, install_neuronx_cc_hook

    install_neuronx_cc_hook()
    nc = build_bass()

    in_names, out_names, out_avals, zero_outs = [], [], [], []
    for alloc in nc.m.functions[0].allocations:
        if not isinstance(alloc, mybir.MemoryLocationSet):
            continue
        name = alloc.memorylocations[0].name
        if alloc.kind == "ExternalInput":
            in_names.append(name)
        elif alloc.kind == "ExternalOutput":
            out_names.append(name)
            shape = tuple(alloc.tensor_shape)
            dtype = mybir.dt.np(alloc.dtype)
            out_avals.append(jax.core.ShapedArray(shape, dtype))
            zero_outs.append(np.zeros(shape, dtype))
    n_params = len(in_names)
    all_names = in_names + out_names

    def _body(*args):
        outs = _bass_exec_p.bind(
            *args,
            out_avals=tuple(out_avals),
            in_names=tuple(all_names),
            out_names=tuple(out_names),
            lowering_input_output_aliases=(),
            sim_require_finite=True,
            sim_require_nnan=True,
            nc=nc,
        )
        return tuple(outs)

    devices = jax.devices()[:NCORES]
    mesh = Mesh(np.asarray(devices), ("core",))
    n_all = n_params + len(out_names)
    sharded = jax.jit(
        shard_map(
            _body, mesh=mesh,
            in_specs=(PartitionSpec("core"),) * n_all,
            out_specs=(PartitionSpec("core"),) * len(out_names),
            check_rep=False,
        ),
        keep_unused=True,
    )
    _CACHE["nc_obj"] = nc
    _CACHE["runner"] = (sharded, in_names, out_names, out_avals, zero_outs)
    return _CACHE["runner"]


def _run_device(in_maps):
    import jax
    sharded, in_names, out_names, out_avals, zero_outs = _get_runner()
    concat_in = [
        np.concatenate([in_maps[c][n] for c in range(NCORES)], axis=0)
        for n in in_names
    ]
    concat_zero = [
        np.zeros((NCORES * z.shape[0], *z.shape[1:]), z.dtype)
        for z in zero_outs
    ]
    args = [jax.device_put(a) for a in concat_in + concat_zero]
    _CACHE["last_args"] = args
    out_arrs = sharded(*args)
    out_arrs = [np.asarray(o) for o in out_arrs]
    return [
        {n: out_arrs[i].reshape(NCORES, *out_avals[i].shape)[c]
         for i, n in enumerate(out_names)}
        for c in range(NCORES)
    ]


def bench(n=10):
    """Re-run the cached jitted fn on the last inputs; returns per-call
    wall seconds. Includes dispatch/tunnel overhead."""
    import time as _time
    sharded = _CACHE["runner"][0]
    args = _CACHE["last_args"]
    times = []
    for _ in range(n):
        t0 = _time.perf_counter()
        res = sharded(*args)
        for r in res:
            r.block_until_ready()
        times.append(_time.perf_counter() - t0)
    return times


def kernel(x, attn_mask, alibi_bias, Wqkv, Wout):
    in_maps = prep_inputs(x, attn_mask, alibi_bias, Wqkv, Wout)
    results = _run_device(in_maps)
    acc = results[0]["out"].astype(np.float32).copy()
    for c in range(1, NCORES):
        acc += results[c]["out"]
    return acc.reshape(B, T, C)



def bench_async(ks=(1, 8, 16), n=4):
    """Queue k async dispatches of the cached jitted fn, block once.
    Marginal device time ~ (T(k2) - T(k1)) / (k2 - k1)."""
    import time as _time
    sharded = _CACHE["runner"][0]
    args = _CACHE["last_args"]
    out = {}
    for k in ks:
        best = float("inf")
        for _ in range(n):
            t0 = _time.perf_counter()
            rs = []
            for _i in range(k):
                rs.append(sharded(*args))
            for x in rs[-1]:
                x.block_until_ready()
            best = min(best, _time.perf_counter() - t0)
        out[k] = best
    return out
